# revision 21
# baseline (speedup 1.0000x reference)
# Causal gated D2 (linear) attention — Trainium2 Bass kernel, 8 NeuronCores.
#
# Sharding: core c -> batch b = c // 4, head group g = c % 4 (heads 3g..3g+2).
# Each core computes LN-stats, qkv/gate projections, chunked causal linear
# attention for its 3 heads, and a partial output projection. Host sums the
# 4 partial proj outputs per batch and assembles the gate output.
#
# Numerics: the qkv/gate projections and LN stats run in float32r (TF32-like,
# ~7e-5 rel err, full PE rate at moving-dim >= 256). The attention internals
# (q/k/v post-LN, chunk state, output projection) run in bf16 with fp32 PSUM
# accumulation. The gate output is computed and returned in fp32.
#
# LayerNorm is folded into an affine post-correction of the qkv matmul:
#   qkv = rstd * (x @ W_eff) - (rstd*mu) * colsum(W_eff),
# W_eff = diag(ln_g) @ W_qkv (folded on host). Stats (rstd, rstd*mu) are
# computed redundantly on all 128 partitions via ones-matmuls so no partition
# broadcasts are needed. Reciprocals run on ACT as exp(-ln(x)).

import numpy as np

B, L, D, H = 2, 1024, 768, 12
DH = 64
NCORES = 8
HPC = 3          # heads per core
GROUPS = 4       # head groups per batch
CHUNK = 256      # attention chunk length
NCH = L // CHUNK

_compiled = {}


def _split_drain_tile_context():
    """TileContext that caps sem waits per instruction (this walrus build
    rejects instructions carrying more than one sync wait on several
    instruction templates) by moving excess waits onto preceding
    same-engine nops, and splits the tail drain the same way."""
    import concourse.tile as tile
    import concourse.mybir as mybir
    from concourse.tile import ScopedClock
    import bass_rust

    MAXW = 1

    class SplitDrainTileContext(tile.TileContext):
        _wsplit_counter = 0

        def _lower_ordered_insts(self, ordered):
            for bb_name, insts in ordered.items():
                need = any(
                    getattr(i, "sync_info", None) is not None
                    and len(i.sync_info.on_wait) > MAXW
                    for i in insts
                )
                if not need:
                    continue
                new_list = []
                for inst in insts:
                    si = getattr(inst, "sync_info", None)
                    if si is not None and len(si.on_wait) > MAXW:
                        waits = list(si.on_wait)
                        si.on_wait.clear()
                        for w in waits[MAXW:]:
                            SplitDrainTileContext._wsplit_counter += 1
                            nop = mybir.InstNoOp(
                                name=f"__wsplit_{SplitDrainTileContext._wsplit_counter}",
                                ins=[], outs=[],
                            )
                            nop.engine = inst.engine
                            nop.sync_info = bass_rust.SyncInfo(
                                on_wait=[w], on_update=[]
                            )
                            new_list.append(nop)
                        for w in waits[:MAXW]:
                            si.on_wait.append(w)
                    new_list.append(inst)
                insts[:] = new_list
            return super()._lower_ordered_insts(ordered)

        def _drain_and_barrier(self, tick_clock, wait_clock):
            drain_inst = self.nc.sync.drain()
            wait_clock.add_sem_waits(
                drain_inst.ins, ScopedClock({None: tick_clock.global_clock})
            )
            si = drain_inst.ins.sync_info
            if si is not None and len(si.on_wait) > 1:
                waits = list(si.on_wait)
                si.on_wait.clear()
                si.on_wait.append(waits[0])
                for w in waits[1:]:
                    d2 = self.nc.sync.drain()
                    si2 = d2.ins.sync_info
                    if si2 is None:
                        d2.ins.sync_info = bass_rust.SyncInfo(
                            on_wait=[w], on_update=[]
                        )
                    else:
                        si2.on_wait.append(w)
            self.nc.all_engine_barrier()
            assert self.sems is not None
            popped = self.nc._tile_sem_poison_stack.pop()
            assert popped is self._sem_poison
            self.nc.clear_and_free_semaphores(list(self.sems.allocated().values()))
            self.nc.all_engine_barrier()

    return SplitDrainTileContext


# Per-head placement inside the packed 6-tile feature layout.
# wAll columns (6 tiles of 128):
#   t0 = [q0 | q1]   t1 = [k0 | k1]   t2 = [g0 | g1]
#   t3 = [q2 | v0]   t4 = [k2 | v1]   t5 = [g2 | v2]
# q_h, k_h, g_h share a partition base per head; v is always at base 64.
Q_POS = [(0, 0), (0, 64), (3, 0)]     # (tile, row) per head
K_POS = [(1, 0), (1, 64), (4, 0)]
G_POS = [(2, 0), (2, 64), (5, 0)]
V_POS = [(3, 64), (4, 64), (5, 64)]


def _build_nc(has_qkv_bias, has_gate_bias):
    import concourse.bass as bass
    import concourse.mybir as mybir

    f32 = mybir.dt.float32
    f32r = mybir.dt.float32r
    bf16 = mybir.dt.bfloat16
    Alu = mybir.AluOpType
    Act = mybir.ActivationFunctionType

    TC = _split_drain_tile_context()

    nc = bass.Bass()
    # ---- DRAM I/O ----
    xT = nc.dram_tensor("xT", (D, L), f32r, kind="ExternalInput")
    wAll = nc.dram_tensor("wAll", (D, 768), f32r, kind="ExternalInput")
    negcs = nc.dram_tensor("negcs", (6, 128), f32, kind="ExternalInput")
    wP = nc.dram_tensor("wP", (HPC * DH, D), bf16, kind="ExternalInput")
    onesR = nc.dram_tensor("onesR", (128, 128), f32r, kind="ExternalInput")
    onesB = nc.dram_tensor("onesB", (128, 192), bf16, kind="ExternalInput")
    maskI = nc.dram_tensor("maskI", (128, 384), bf16, kind="ExternalInput")
    if has_qkv_bias:
        qkvbI = nc.dram_tensor("qkvbI", (6, 128), f32, kind="ExternalInput")
    if has_gate_bias:
        gbI = nc.dram_tensor("gbI", (6, 128), f32, kind="ExternalInput")
    outP = nc.dram_tensor("outP", (L, D), f32, kind="ExternalOutput")
    gateT = nc.dram_tensor("gateT", (HPC * DH, L), f32, kind="ExternalOutput")

    with TC(nc) as tc:
        with tc.tile_pool(name="const", bufs=1) as cp:
            onesR_sb = cp.tile([128, 128], f32r)
            nc.sync.dma_start(onesR_sb[:], onesR[:])
            xT_sb = cp.tile([128, 6, L], f32r)
            xTr = xT.rearrange("(ks p) l -> p ks l", p=128)
            wAll_sb = cp.tile([128, 6, 768], f32r)
            wAr = wAll.rearrange("(ks p) m -> p ks m", p=128)
            for ks in range(6):
                nc.sync.dma_start(xT_sb[:, ks], xTr[:, ks])
                nc.sync.dma_start(wAll_sb[:, ks], wAr[:, ks])
            wp_sb = cp.tile([64, HPC, D], bf16)
            nc.sync.dma_start(wp_sb[:], wP.rearrange("(h p) n -> p h n", p=64))
            onesB_sb = cp.tile([128, 192], bf16)
            nc.sync.dma_start(onesB_sb[:], onesB[:])
            mask_sb = cp.tile([128, 384], bf16)
            nc.sync.dma_start(mask_sb[:], maskI[:])
            negcs_sb = cp.tile([128, 6], f32)
            nc.sync.dma_start(negcs_sb[:], negcs.rearrange("m p -> p m"))
            if has_qkv_bias:
                qkvb_sb = cp.tile([128, 6], f32)
                nc.sync.dma_start(qkvb_sb[:], qkvbI.rearrange("m p -> p m"))
            if has_gate_bias:
                gb_sb = cp.tile([128, 6], f32)
                nc.sync.dma_start(gb_sb[:], gbI.rearrange("m p -> p m"))

            epsb = cp.tile([128, 2], f32)         # [:,0]=1e-5 (LN), [:,1]=1e-6
            nc.vector.memset(epsb[:, 0:1], 1e-5)
            nc.vector.memset(epsb[:, 1:2], 1e-6)

            QK = cp.tile([128, 6, L], bf16)       # corrected q/k/v store
            gf = cp.tile([128, 2, L], f32)        # gate (fp32): [g0|g1], [g2|-]
            attnT = cp.tile([64, HPC, L], bf16)   # attention out (pre-proj), T
            rstdB = cp.tile([128, L], f32)
            rmuB = cp.tile([128, L], f32)
            # t5-lower is never written (gate lives in gf) but its columns go
            # through the DMA transpose; zero once so it's not uninitialized.
            nc.vector.memset(QK[0:64, 5, :], 0.0)
            nc.vector.memset(QK[:, 2, :], 0.0)

            # nat tiles (position-major K/V for the attention chunk
            # matmuls), produced by DMA transposes as soon as each 512-token
            # block of QK is finalized in phase 1.
            # Layout per (chunk, 128-block): [128, 7, 128]:
            #   s0 = [k0n|k1n]  s1 = [q2n|v0n]  s2 = [ones|-]
            #   s3 = [k2n|v1n]  s4 = [ones|-]   s5 = [.|v2n]  s6 = [ones|-]
            TRN_SLOTS = {1: 0, 3: 1, 4: 3, 5: 5}
            KN_POS = [(0, 0), (0, 64), (3, 0)]   # (slot, col) of k_nat
            V_SLOT = [1, 3, 5]                   # slot with v_h at cols 64:

            def vaug_ap(nat, h):
                # [v_h | ones]: 128 contiguous columns starting at v_h
                v = nat[:, V_SLOT[h], 64:128]
                return bass.AP(
                    tensor=v.tensor, offset=v.offset,
                    ap=[v.ap[0], [1, 128]],
                )

            natp = cp  # persistent tiles, one per (chunk, block)
            onesB3 = onesB_sb[:].rearrange("p (s x) -> p s x", s=3)
            nats_all = {}

            def make_nats(c, trn_i=[0]):
                for jb in range(2):
                    psl = slice(CHUNK * c + 128 * jb,
                                CHUNK * c + 128 * jb + 128)
                    nat = natp.tile([128, 7, 128], bf16, tag=f"nat{c}_{jb}",
                                    name=f"nat{c}_{jb}")
                    for (m, s) in TRN_SLOTS.items():
                        eng = nc.scalar if trn_i[0] % 2 else nc.sync
                        trn_i[0] += 1
                        eng.dma_start(
                            nat[:, s, :], QK[:, m, psl], transpose=True
                        )
                    o1 = nat[:, 2, 0:64]
                    ones_dst = bass.AP(
                        tensor=o1.tensor, offset=o1.offset,
                        ap=[o1.ap[0], [256, 3], o1.ap[1]],
                    )
                    nc.gpsimd.tensor_copy(out=ones_dst, in_=onesB3)
                    nats_all[(c, jb)] = nat

            # ---------- phase 1: LN stats + qkv/gate projections ----------
            with tc.tile_pool(name="x2p", bufs=1) as x2p, \
                 tc.tile_pool(name="stps", bufs=4, space="PSUM") as stps, \
                 tc.tile_pool(name="zps", bufs=3, space="PSUM") as zps, \
                 tc.tile_pool(name="wk", bufs=3) as wk:
                x2 = x2p.tile([128, 6, L], f32r)
                for ks in range(6):
                    nc.gpsimd.tensor_tensor(
                        x2[:, ks], xT_sb[:, ks], xT_sb[:, ks], op=Alu.mult
                    )
                sum_ps = [stps.tile([128, 512], f32, tag="st", name=f"sum{i}")
                          for i in range(2)]
                ssq_ps = [stps.tile([128, 512], f32, tag="st", name=f"ssq{i}")
                          for i in range(2)]
                for lc in range(2):
                    sl = slice(512 * lc, 512 * lc + 512)
                    for ks in range(6):
                        nc.tensor.matmul(
                            sum_ps[lc][:], lhsT=onesR_sb[:],
                            rhs=xT_sb[:, ks, sl],
                            start=(ks == 0), stop=(ks == 5),
                        )
                    for ks in range(6):
                        nc.tensor.matmul(
                            ssq_ps[lc][:], lhsT=onesR_sb[:],
                            rhs=x2[:, ks, sl],
                            start=(ks == 0), stop=(ks == 5),
                        )
                for lc in range(2):
                    sl = slice(512 * lc, 512 * lc + 512)
                    mu = wk.tile([128, 512], f32, tag="mu")
                    nc.vector.tensor_scalar_mul(out=mu[:], in0=sum_ps[lc][:], scalar1=1.0 / D)
                    msq = wk.tile([128, 512], f32, tag="msq")
                    nc.vector.tensor_tensor(msq[:], mu[:], mu[:], op=Alu.mult)
                    var = wk.tile([128, 512], f32, tag="var")
                    nc.vector.scalar_tensor_tensor(
                        var[:], in0=ssq_ps[lc][:], scalar=1.0 / D, in1=msq[:],
                        op0=Alu.mult, op1=Alu.subtract,
                    )
                    # rstd = exp(-0.5 * ln(var + eps))
                    nc.scalar.activation(
                        out=var[:], in_=var[:], func=Act.Ln, bias=epsb[:, 0:1]
                    )
                    nc.scalar.activation(
                        out=rstdB[:, sl], in_=var[:], func=Act.Exp, scale=-0.5
                    )
                    nc.vector.tensor_tensor(
                        rmuB[:, sl], rstdB[:, sl], mu[:], op=Alu.mult
                    )

                # qkv/gate matmuls + per-tile corrections
                for lc in range(2):
                    sl = slice(512 * lc, 512 * lc + 512)
                    for m in range(6):
                        z = zps.tile([128, 512], f32, tag="z")
                        for ks in range(6):
                            nc.tensor.matmul(
                                z[:], lhsT=wAll_sb[:, ks, 128 * m:128 * m + 128],
                                rhs=xT_sb[:, ks, sl],
                                start=(ks == 0), stop=(ks == 5),
                            )

                        def corr(rows, mm=m, zz=z, ssl=sl):
                            tq = wk.tile([128, 512], f32, tag="tq")
                            nc.vector.tensor_tensor(
                                tq[rows], zz[rows], rstdB[rows, ssl], op=Alu.mult
                            )
                            if has_qkv_bias:
                                nc.vector.scalar_tensor_tensor(
                                    tq[rows], in0=rmuB[rows, ssl],
                                    scalar=negcs_sb[rows, mm:mm + 1],
                                    in1=tq[rows], op0=Alu.mult, op1=Alu.add,
                                )
                                nc.vector.tensor_scalar_add(
                                    out=QK[rows, mm, ssl], in0=tq[rows],
                                    scalar1=qkvb_sb[rows, mm:mm + 1],
                                )
                            else:
                                nc.vector.scalar_tensor_tensor(
                                    QK[rows, mm, ssl], in0=rmuB[rows, ssl],
                                    scalar=negcs_sb[rows, mm:mm + 1],
                                    in1=tq[rows], op0=Alu.mult, op1=Alu.add,
                                )

                        def sigm(rows, gsl, mm=m, zz=z, ssl=sl):
                            gbias = gb_sb[rows, mm:mm + 1] if has_gate_bias else 0.0
                            nc.scalar.activation(
                                out=gsl, in_=zz[rows],
                                func=Act.Sigmoid, bias=gbias,
                            )

                        if m in (0, 1, 3, 4):
                            corr(slice(0, 128))
                        elif m == 2:
                            sigm(slice(0, 128), gf[:, 0, sl])
                        else:  # m == 5: lower = gate g2, upper = v2
                            sigm(slice(0, 64), gf[0:64, 1, sl])
                            corr(slice(64, 128))

                    # k := k * gate, then elu+1 on q and k (in place)
                    nc.vector.tensor_tensor(
                        QK[:, 1, sl], QK[:, 1, sl], gf[:, 0, sl], op=Alu.mult
                    )
                    nc.vector.tensor_tensor(
                        QK[0:64, 4, sl], QK[0:64, 4, sl], gf[0:64, 1, sl],
                        op=Alu.mult,
                    )
                    for (m, rows) in ((0, slice(0, 128)), (1, slice(0, 128)),
                                      (3, slice(0, 64)), (4, slice(0, 64))):
                        tmin = wk.tile([128, 512], bf16, tag="tmin")
                        nc.vector.tensor_scalar_min(
                            out=tmin[rows], in0=QK[rows, m, sl], scalar1=0.0,
                        )
                        texp = wk.tile([128, 512], bf16, tag="texp")
                        nc.scalar.activation(
                            out=texp[rows], in_=tmin[rows], func=Act.Exp
                        )
                        nc.vector.scalar_tensor_tensor(
                            QK[rows, m, sl], in0=QK[rows, m, sl], scalar=0.0,
                            in1=texp[rows], op0=Alu.max, op1=Alu.add,
                        )
                    make_nats(2 * lc)
                    make_nats(2 * lc + 1)

            # gate output DMA (transposed layout; host transposes back)
            nc.sync.dma_start(gateT[0:128, :], gf[:, 0, :])
            nc.sync.dma_start(gateT[128:192, :], gf[0:64, 1, :])

            # ---------- phase 2: chunked causal linear attention ----------
            # Per (chunk, 128-block): DMA-transpose four QK feature tiles into
            # a nat tile [128, 7, 128] with ones blocks interleaved so each
            # head's [v_h | ones] pair is one contiguous 128-column run:
            #   s0 = [k0n|k1n]  s1 = [q2n|v0n]  s2 = [ones|-]
            #   s3 = [k2n|v1n]  s4 = [ones|-]   s5 = [.|v2n]  s6 = [ones|-]
            # The ones columns produce the attention denominator rows.
            with tc.tile_pool(name="aps", bufs=2, space="PSUM") as aps, \
                 tc.tile_pool(name="ops", bufs=2, space="PSUM") as ops, \
                 tc.tile_pool(name="sps", bufs=2, space="PSUM") as sps, \
                 tc.tile_pool(name="pps", bufs=2, space="PSUM") as pps, \
                 tc.tile_pool(name="asb", bufs=4) as asb, \
                 tc.tile_pool(name="ssb", bufs=6) as ssb, \
                 tc.tile_pool(name="dnp", bufs=4) as dnp, \
                 tc.tile_pool(name="posb", bufs=3) as posb:
                S_prev = [None] * HPC
                for c in range(NCH):
                    cs = slice(CHUNK * c, CHUNK * (c + 1))
                    nats = [nats_all[(c, 0)], nats_all[(c, 1)]]

                    for h in range(HPC):
                        qm, qr = Q_POS[h]
                        km, kr = K_POS[h]
                        qsl = QK[qr:qr + 64, qm, cs]       # [64, 256]
                        A_sb = []
                        for jb in range(2):
                            psl = slice(CHUNK * c + 128 * jb,
                                        CHUNK * c + 128 * jb + 128)
                            a_ps = aps.tile([128, CHUNK], f32, tag="a")
                            nc.tensor.matmul(
                                a_ps[:], lhsT=QK[kr:kr + 64, km, psl],
                                rhs=qsl, start=True, stop=True,
                            )
                            a_s = asb.tile([128, CHUNK], bf16, tag="asb")
                            msl = (mask_sb[:, 128:384] if jb == 0
                                   else mask_sb[:, 0:256])
                            nc.vector.tensor_tensor(
                                a_s[:], a_ps[:], msl, op=Alu.mult
                            )
                            A_sb.append(a_s)

                        o_ps = ops.tile([128, CHUNK], f32, tag="o")
                        first = True
                        if c > 0:
                            nc.tensor.matmul(
                                o_ps[:], lhsT=S_prev[h], rhs=qsl,
                                start=True, stop=False,
                            )
                            first = False
                        for jb in range(2):
                            nc.tensor.matmul(
                                o_ps[:], lhsT=vaug_ap(nats[jb], h),
                                rhs=A_sb[jb][:],
                                start=first, stop=(jb == 1),
                            )
                            first = False

                        s_ps = sps.tile([64, 128], f32, tag="s")
                        for jb in range(2):
                            ks_, kc = KN_POS[h]
                            nc.tensor.matmul(
                                s_ps[:],
                                lhsT=nats[jb][:, ks_, kc:kc + 64],
                                rhs=vaug_ap(nats[jb], h),
                                start=(jb == 0), stop=(jb == 1),
                            )
                        if c < NCH - 1:
                            s_big = ssb.tile([128, 128], bf16, tag="ssb")
                            s_new = s_big[kr:kr + 64, :]
                            if c == 0:
                                nc.vector.tensor_copy(out=s_new, in_=s_ps[:])
                            else:
                                nc.vector.tensor_tensor(
                                    s_new, s_ps[:], S_prev[h], op=Alu.add
                                )
                            S_prev[h] = s_new

                        # attn = O_num * exp(-ln(den + 1e-6))
                        den = dnp.tile([64, CHUNK], f32, tag="den")
                        nc.scalar.activation(
                            out=den[:], in_=o_ps[64:128, :],
                            func=Act.Ln, bias=epsb[0:64, 1:2],
                        )
                        rec = dnp.tile([64, CHUNK], f32, tag="rec")
                        nc.scalar.activation(
                            out=rec[:], in_=den[:], func=Act.Exp, scale=-1.0
                        )
                        nc.vector.tensor_tensor(
                            attnT[:, h, cs], o_ps[0:64, :], rec[:], op=Alu.mult
                        )

                    # trailing output projection for this chunk's token tiles
                    for tt in (2 * c, 2 * c + 1):
                        tsl = slice(128 * tt, 128 * tt + 128)
                        for nb in range(2):
                            nsl = slice(384 * nb, 384 * nb + 384)
                            p_ps = pps.tile([128, 384], f32, tag="p")
                            for h in range(HPC):
                                nc.tensor.matmul(
                                    p_ps[:], lhsT=attnT[:, h, tsl],
                                    rhs=wp_sb[:, h, nsl],
                                    start=(h == 0), stop=(h == HPC - 1),
                                )
                            po = posb.tile([128, 384], f32, tag="po")
                            nc.vector.tensor_copy(out=po[:], in_=p_ps[:])
                            nc.gpsimd.dma_start(outP[tsl, nsl], po[:])
    return nc


def _get_compiled(has_qkv_bias, has_gate_bias):
    key = (has_qkv_bias, has_gate_bias)
    if key not in _compiled:
        _compiled[key] = _build_nc(has_qkv_bias, has_gate_bias)
    return _compiled[key]


def _host_prep(x, W_qkv, b_qkv, W_gate, b_gate, W_proj, b_proj, ln_g, ln_b):
    """Build the 8 per-core input maps."""
    import ml_dtypes

    x = np.ascontiguousarray(np.asarray(x, np.float32))
    W_qkv = np.asarray(W_qkv, np.float32)
    W_gate = np.asarray(W_gate, np.float32)
    W_proj = np.asarray(W_proj, np.float32)
    ln_g = np.asarray(ln_g, np.float32)
    ln_b = np.asarray(ln_b, np.float32)
    b_qkv = np.asarray(b_qkv, np.float32)
    b_gate = np.asarray(b_gate, np.float32)

    W_eff = W_qkv * ln_g[:, None]
    # bias row folded through the LN affine: ln_b @ W_qkv + b_qkv
    qkv_bias_row = ln_b @ W_qkv + b_qkv

    mask = np.zeros((128, 384), np.float32)
    p = np.arange(128)[:, None]
    cidx = np.arange(384)[None, :]
    mask[(p <= cidx - 128)] = 1.0
    mask = mask.astype(ml_dtypes.bfloat16)
    onesR = np.ones((128, 128), np.float32)
    onesB = np.ones((128, 192), ml_dtypes.bfloat16)

    in_maps = []
    for c in range(NCORES):
        b = c // GROUPS
        g = c % GROUPS
        hs = slice(192 * g, 192 * g + 192)
        Wq = W_eff[:, 0:768][:, hs]
        Wk = W_eff[:, 768:1536][:, hs]
        Wv = W_eff[:, 1536:2304][:, hs]
        Wg = W_gate[:, hs]
        bq = qkv_bias_row[0:768][hs]
        bk = qkv_bias_row[768:1536][hs]
        bv = qkv_bias_row[1536:2304][hs]
        bg = b_gate[hs]

        tiles = [
            np.concatenate([Wq[:, 0:64], Wq[:, 64:128]], axis=1),
            np.concatenate([Wk[:, 0:64], Wk[:, 64:128]], axis=1),
            np.concatenate([Wg[:, 0:64], Wg[:, 64:128]], axis=1),
            np.concatenate([Wq[:, 128:192], Wv[:, 0:64]], axis=1),
            np.concatenate([Wk[:, 128:192], Wv[:, 64:128]], axis=1),
            np.concatenate([Wg[:, 128:192], Wv[:, 128:192]], axis=1),
        ]
        wAll = np.concatenate(tiles, axis=1)  # (768, 768)

        # negated column sums (LN correction), zero for gate columns
        negcs = np.zeros((6, 128), np.float32)
        qkvb = np.zeros((6, 128), np.float32)
        gateb = np.zeros((6, 128), np.float32)
        cs_q = Wq.sum(0); cs_k = Wk.sum(0); cs_v = Wv.sum(0)
        for h in range(HPC):
            mq, rq = Q_POS[h]; negcs[mq, rq:rq + 64] = -cs_q[64 * h:64 * h + 64]
            mk, rk = K_POS[h]; negcs[mk, rk:rk + 64] = -cs_k[64 * h:64 * h + 64]
            mv, rv = V_POS[h]; negcs[mv, rv:rv + 64] = -cs_v[64 * h:64 * h + 64]
            qkvb[mq, rq:rq + 64] = bq[64 * h:64 * h + 64]
            qkvb[mk, rk:rk + 64] = bk[64 * h:64 * h + 64]
            qkvb[mv, rv:rv + 64] = bv[64 * h:64 * h + 64]
            mg, rg = G_POS[h]; gateb[mg, rg:rg + 64] = bg[64 * h:64 * h + 64]

        in_maps.append({
            "xT": np.ascontiguousarray(x[b].T),
            "wAll": np.ascontiguousarray(wAll),
            "negcs": negcs,
            "wP": np.ascontiguousarray(W_proj[hs, :]).astype(ml_dtypes.bfloat16),
            "onesR": onesR,
            "onesB": onesB,
            "maskI": mask,
            "_qkvb": qkvb,
            "_gateb": gateb,
        })
    return in_maps


def _finalize_in_maps(in_maps):
    has_qkv_bias = any(np.any(m["_qkvb"]) for m in in_maps)
    has_gate_bias = any(np.any(m["_gateb"]) for m in in_maps)
    for m in in_maps:
        qb = m.pop("_qkvb")
        gb = m.pop("_gateb")
        if has_qkv_bias:
            m["qkvbI"] = qb
        if has_gate_bias:
            m["gbI"] = gb
    return has_qkv_bias, has_gate_bias


def _assemble(results, b_proj):
    b_proj = np.asarray(b_proj, np.float32)
    out = np.zeros((B, L, D), np.float32)
    gate = np.zeros((B, L, D), np.float32)
    for c in range(NCORES):
        b = c // GROUPS
        g = c % GROUPS
        r = results[c]
        out[b] += r["outP"]
        gate[b][:, 192 * g:192 * g + 192] = r["gateT"].T
    out += b_proj
    return out, gate


def kernel(x, W_qkv, b_qkv, W_gate, b_gate, W_proj, b_proj, ln_g, ln_b):
    import concourse.bass_utils as bass_utils

    in_maps = _host_prep(x, W_qkv, b_qkv, W_gate, b_gate, W_proj, b_proj,
                         ln_g, ln_b)
    has_qkv_bias, has_gate_bias = _finalize_in_maps(in_maps)
    nc = _get_compiled(has_qkv_bias, has_gate_bias)
    res = bass_utils.run_bass_kernel_spmd(
        nc, in_maps, core_ids=list(range(NCORES))
    )
    return _assemble(res.results, b_proj)
